# revision 8
# baseline (speedup 1.0000x reference)
"""Trainium2 Bass kernel for the CgpHmm scaled-forward layer.

Computes loglik[b] = scaled HMM forward log-likelihood over B=128 sequences
of length T=8192 with S=128 hidden states and an alphabet of E=6 symbols.

Strategy (data-parallel over batch, 16 sequences per core on 8 cores):
  - Keep alphaT [S=128 partitions, 16 seqs] resident in SBUF.
  - Per time step: PE matmul (stationary A, 16 moving cols) computes
    A^T @ alphaT into PSUM; one DVE tensor_mul multiplies by the emission
    column (streamed from HBM) and writes the next alphaT back to SBUF.
    This 2-instruction serial chain (8192 links) is latency-bound and is
    the entire kernel critical path.
  - Emission probabilities are pre-divided on the host by a per-symbol
    predictor f_sym[e] = (stationary dist of A) @ Bm[:, e] so that alpha's
    magnitude drifts slowly; exact per-sequence renormalization happens
    every 128 steps via a ones-vector matmul (Z), reciprocal, and a lagged
    rescale 16 steps later.  All log bookkeeping (sum of log Z, log f
    counts, final log-sum) happens on the host in float64.
"""

import os
import sys

import numpy as np

sys.path.insert(0, "/opt/trn_rl_repo")

P = 128          # states / partitions
BL = 16          # sequences per core
N_CORES = 8
B_FULL = 128
T_FULL = 8192
E_SYM = 6

# device schedule constants (full size)
STEPS_PER_BODY = 512      # chain steps per For_i iteration
CHUNK_STEPS = 256         # steps per emission SBUF tile (2 tiles per body)
RENORM_EVERY = 128
RENORM_LAG = 16


def build_nc(n_iters, steps_per_body, chunk_steps, renorm_every, renorm_lag,
             debug=False):
    """Build the per-core Bass program (identical for all cores)."""
    import concourse.bacc as bacc
    import concourse.bass as bass
    import concourse.mybir as mybir
    import concourse.tile as tile

    assert steps_per_body == 2 * chunk_steps
    assert steps_per_body % renorm_every == 0
    n_renorm_per_body = steps_per_body // renorm_every
    n_renorms = n_iters * n_renorm_per_body
    chunk_cols = chunk_steps * BL
    # chunks used: 2*n_iters, plus one pad chunk for the final em0 prefetch
    em_cols = (2 * n_iters + 1) * chunk_cols

    nc = bacc.Bacc(None, target_bir_lowering=False, debug=debug)

    f32 = mybir.dt.float32
    em_d = nc.dram_tensor("em", [P, em_cols], f32, kind="ExternalInput")
    a_d = nc.dram_tensor("amat", [P, P], f32, kind="ExternalInput")
    ones_d = nc.dram_tensor("ones", [P, P], f32, kind="ExternalInput")
    alpha0_d = nc.dram_tensor("alpha0", [P, BL], f32, kind="ExternalInput")
    afin_d = nc.dram_tensor("alpha_fin", [P, BL], f32, kind="ExternalOutput")
    zbuf_d = nc.dram_tensor("zbuf", [1, max(n_renorms, 1) * BL], f32,
                            kind="ExternalOutput")

    with tile.TileContext(nc) as tc, \
            tc.tile_pool(name="sb", bufs=1) as sbp, \
            tc.tile_pool(name="ps", bufs=1, space="PSUM") as psp:
        a_sb = sbp.tile([P, P], f32, name="a_sb")
        ones_sb = sbp.tile([P, P], f32, name="ones_sb")
        al_a = sbp.tile([P, BL], f32, name="al_a")
        al_b = sbp.tile([P, BL], f32, name="al_b")
        em0 = sbp.tile([P, chunk_cols], f32, name="em0")
        em1 = sbp.tile([P, chunk_cols], f32, name="em1")
        rz = sbp.tile([P, BL], f32, name="rz")
        zring = sbp.tile([1, max(n_renorms, 1) * BL], f32, name="zring")
        pp0 = psp.tile([P, BL], f32, name="pp0")
        pp1 = psp.tile([P, BL], f32, name="pp1")
        zps = psp.tile([P, BL], f32, name="zps")

        # preamble loads
        nc.sync.dma_start(a_sb[:], a_d[:])
        nc.sync.dma_start(ones_sb[:], ones_d[:])
        nc.sync.dma_start(al_a[:], alpha0_d[:])
        # chunk 0 preload (body 0 reads it as em0)
        n_dma_slices = 8
        slice_cols = chunk_cols // n_dma_slices
        for s in range(n_dma_slices):
            nc.sync.dma_start(em0[:, s * slice_cols:(s + 1) * slice_cols],
                              em_d[:, s * slice_cols:(s + 1) * slice_cols])

        alphas = (al_a, al_b)
        psums = (pp0, pp1)

        with tc.For_i(0, n_iters, 1) as it:
            # prefetch em1 <- chunk 2i+1 (read in this body's second half)
            base = it * (2 * chunk_cols)
            for s in range(n_dma_slices):
                off = s * slice_cols
                nc.sync.dma_start(
                    em1[:, off:off + slice_cols],
                    em_d[:, bass.ds(base + chunk_cols + off, slice_cols)])

            for u in range(steps_per_body):
                cur = alphas[u % 2]
                nxt = alphas[(u + 1) % 2]
                ps = psums[u % 2]
                emt = em0 if u < chunk_steps else em1
                c0 = (u if u < chunk_steps else u - chunk_steps) * BL
                # chain: psum <- A^T @ alpha ; alpha' <- psum * em[:, step]
                nc.tensor.matmul(ps[:], a_sb[:], cur[:])
                nc.vector.tensor_mul(nxt[:], ps[:], emt[:, c0:c0 + BL])

                if u % renorm_every == 0:
                    # Z = colsum of the alpha just produced (off-chain-ish)
                    m = u // renorm_every
                    nc.tensor.matmul(zps[:], ones_sb[:], nxt[:])
                    nc.vector.reciprocal(rz[:], zps[:])
                    nc.scalar.copy(
                        zring[0:1, bass.ds((it * n_renorm_per_body + m) * BL,
                                           BL)],
                        zps[0:1, :])
                if u % renorm_every == renorm_lag:
                    # lagged rescale: alpha *= 1/Z (broadcast over partitions)
                    nc.vector.tensor_mul(nxt[:], nxt[:], rz[:])

                if u == chunk_steps - 1:
                    # em0's readers are done; prefetch em0 <- chunk 2i+2
                    for s in range(n_dma_slices):
                        off = s * slice_cols
                        nc.sync.dma_start(
                            em0[:, off:off + slice_cols],
                            em_d[:, bass.ds(base + 2 * chunk_cols + off,
                                            slice_cols)])

        # steps_per_body is even, so the final alpha is in al_a
        nc.sync.dma_start(afin_d[:], al_a[:])
        nc.sync.dma_start(zbuf_d[:], zring[:])

    nc.compile()
    return nc


def host_prepare(obs, I, A, Bm, n_iters=None):
    """Shard + precompute per-core device inputs and host bookkeeping."""
    if n_iters is None:
        n_iters = T_FULL // STEPS_PER_BODY
    T = n_iters * STEPS_PER_BODY
    chunk_cols = CHUNK_STEPS * BL
    em_cols = (2 * n_iters + 1) * chunk_cols

    obs = np.asarray(obs)
    I64 = np.asarray(I, np.float64)
    A64 = np.asarray(A, np.float64)
    Bm64 = np.asarray(Bm, np.float64)

    # stationary distribution of A -> per-symbol magnitude predictor
    pi = np.full(P, 1.0 / P)
    for _ in range(300):
        pi = pi @ A64
    f_sym = pi @ Bm64                                   # [E]
    Bmh32 = (Bm64 / f_sym[None, :]).astype(np.float32)  # folded emissions

    ones = np.ones((P, P), np.float32)
    A32 = np.asarray(A, np.float32)

    in_maps = []
    book = []
    for c in range(N_CORES):
        ob = obs[c * BL:(c + 1) * BL]                   # [16, T_total]
        # step 0 on host (float64): alpha0 = I * Bm[:, obs0], normalized
        a0 = I64[:, None] * Bm64[:, ob[:, 0]]           # [S, 16]
        Z0 = a0.sum(0)
        alpha0 = (a0 / Z0).astype(np.float32)
        # emission stream: col j <-> chain step j+1 for j < T-1;
        # col T-1 is all-ones (sum-preserving extra step); pad chunk zeros.
        em = np.empty((P, em_cols), np.float32)
        obsT = ob[:, 1:T].T                             # [T-1, 16]
        em[:, :(T - 1) * BL] = Bmh32[:, obsT.reshape(-1)]
        em[:, (T - 1) * BL:T * BL] = 1.0
        em[:, T * BL:] = 0.0
        in_maps.append({"em": em, "amat": A32, "ones": ones,
                        "alpha0": alpha0})
        # host-side fp64 pieces of the final loglik
        cnt = np.stack([(ob[:, 1:T] == e).sum(1) for e in range(E_SYM)], 1)
        ll_base = np.log(Z0) + (cnt * np.log(f_sym)[None, :]).sum(1)  # [16]
        book.append(ll_base)
    return in_maps, book


def assemble_output(results, book):
    """Combine device outputs + host bookkeeping into loglik [128] f32."""
    out = np.empty(B_FULL, np.float64)
    for c in range(N_CORES):
        afin = results[c]["alpha_fin"].astype(np.float64)   # [S, 16]
        zb = results[c]["zbuf"].astype(np.float64).reshape(-1, BL)  # [R, 16]
        ll = book[c] + np.log(zb).sum(0) + np.log(afin.sum(0))
        out[c * BL:(c + 1) * BL] = ll
    return out.astype(np.float32)


_NC_CACHE = {}


def _get_nc(n_iters):
    key = n_iters
    if key not in _NC_CACHE:
        _NC_CACHE[key] = build_nc(n_iters, STEPS_PER_BODY, CHUNK_STEPS,
                                  RENORM_EVERY, RENORM_LAG)
    return _NC_CACHE[key]


def kernel(obs, I, A, Bm):
    from concourse.bass_utils import run_bass_kernel_spmd

    n_iters = T_FULL // STEPS_PER_BODY
    nc = _get_nc(n_iters)
    in_maps, book = host_prepare(obs, I, A, Bm, n_iters)
    res = run_bass_kernel_spmd(nc, in_maps, core_ids=list(range(N_CORES)))
    return assemble_output(res.results, book)


# revision 10
# speedup vs baseline: 1.9879x; 1.9879x over previous
"""Trainium2 Bass kernel for the CgpHmm scaled-forward layer.

Computes loglik[b] = scaled HMM forward log-likelihood over B=128 sequences
of length T=8192 with S=128 hidden states and an alphabet of E=6 symbols.

Strategy (data-parallel over batch, 16 sequences per core on 8 cores):
  - Keep alphaT [S=128 partitions, 16 seqs] resident in SBUF.
  - Per time step: PE matmul (stationary A, 16 moving cols) computes
    A^T @ alphaT into PSUM; one DVE tensor_mul multiplies by the emission
    column (streamed from HBM) and writes the next alphaT back to SBUF.
    This 2-instruction serial chain (8192 links) is latency-bound and is
    the entire kernel critical path.
  - Emission probabilities are pre-divided on the host by a per-symbol
    predictor f_sym[e] = (stationary dist of A) @ Bm[:, e] so that alpha's
    magnitude drifts slowly; exact per-sequence renormalization happens
    every 128 steps via a ones-vector matmul (Z), reciprocal, and a lagged
    rescale 16 steps later.  All log bookkeeping (sum of log Z, log f
    counts, final log-sum) happens on the host in float64.
"""

import os
import sys

import numpy as np

sys.path.insert(0, "/opt/trn_rl_repo")

P = 128          # states / partitions
BL = 16          # sequences per core
N_CORES = 8
B_FULL = 128
T_FULL = 8192
E_SYM = 6

# device schedule constants (full size)
STEPS_PER_BODY = 512      # chain steps per For_i iteration
CHUNK_STEPS = 256         # steps per emission SBUF tile (2 tiles per body)
RENORM_EVERY = 128
RENORM_LAG = 16
MM_DTYPE = "bfloat16"     # matmul operand dtype ("float32" | "bfloat16")


def build_nc(n_iters, steps_per_body, chunk_steps, renorm_every, renorm_lag,
             debug=False):
    """Build the per-core Bass program (identical for all cores)."""
    import concourse.bacc as bacc
    import concourse.bass as bass
    import concourse.mybir as mybir
    import concourse.tile as tile

    assert steps_per_body == 2 * chunk_steps
    assert steps_per_body % renorm_every == 0
    n_renorm_per_body = steps_per_body // renorm_every
    n_renorms = n_iters * n_renorm_per_body
    chunk_cols = chunk_steps * BL
    # chunks used: 2*n_iters, plus one pad chunk for the final em0 prefetch
    em_cols = (2 * n_iters + 1) * chunk_cols

    nc = bacc.Bacc(None, target_bir_lowering=False, debug=debug)

    f32 = mybir.dt.float32
    mdt = getattr(mybir.dt, MM_DTYPE)
    em_d = nc.dram_tensor("em", [P, em_cols], f32, kind="ExternalInput")
    a_d = nc.dram_tensor("amat", [P, P], mdt, kind="ExternalInput")
    ones_d = nc.dram_tensor("ones", [P, P], mdt, kind="ExternalInput")
    alpha0_d = nc.dram_tensor("alpha0", [P, BL], mdt, kind="ExternalInput")
    afin_d = nc.dram_tensor("alpha_fin", [P, BL], mdt, kind="ExternalOutput")
    zbuf_d = nc.dram_tensor("zbuf", [1, max(n_renorms, 1) * BL], f32,
                            kind="ExternalOutput")

    with tile.TileContext(nc) as tc, \
            tc.tile_pool(name="sb", bufs=1) as sbp, \
            tc.tile_pool(name="ps", bufs=1, space="PSUM") as psp:
        a_sb = sbp.tile([P, P], mdt, name="a_sb")
        ones_sb = sbp.tile([P, P], mdt, name="ones_sb")
        al_a = sbp.tile([P, BL], mdt, name="al_a")
        al_b = sbp.tile([P, BL], mdt, name="al_b")
        em0 = sbp.tile([P, chunk_cols], f32, name="em0")
        em1 = sbp.tile([P, chunk_cols], f32, name="em1")
        rz = sbp.tile([P, BL], f32, name="rz")
        zring = sbp.tile([1, max(n_renorms, 1) * BL], f32, name="zring")
        pp0 = psp.tile([P, BL], f32, name="pp0")
        pp1 = psp.tile([P, BL], f32, name="pp1")
        zps = psp.tile([P, BL], f32, name="zps")

        # preamble loads
        nc.sync.dma_start(a_sb[:], a_d[:])
        nc.sync.dma_start(ones_sb[:], ones_d[:])
        nc.sync.dma_start(al_a[:], alpha0_d[:])
        # chunk 0 preload (body 0 reads it as em0)
        n_dma_slices = 8
        slice_cols = chunk_cols // n_dma_slices
        for s in range(n_dma_slices):
            nc.sync.dma_start(em0[:, s * slice_cols:(s + 1) * slice_cols],
                              em_d[:, s * slice_cols:(s + 1) * slice_cols])

        alphas = (al_a, al_b)
        psums = (pp0, pp1)

        # establish A as the loaded stationary operand (result discarded;
        # zps is overwritten by the first renorm matmul before any read)
        nc.tensor.matmul(zps[:], a_sb[:], al_a[:])

        with tc.For_i(0, n_iters, 1) as it:
            # prefetch em1 <- chunk 2i+1 (read in this body's second half)
            base = it * (2 * chunk_cols)
            for s in range(n_dma_slices):
                off = s * slice_cols
                nc.sync.dma_start(
                    em1[:, off:off + slice_cols],
                    em_d[:, bass.ds(base + chunk_cols + off, slice_cols)])

            for u in range(steps_per_body):
                cur = alphas[u % 2]
                nxt = alphas[(u + 1) % 2]
                ps = psums[u % 2]
                emt = em0 if u < chunk_steps else em1
                c0 = (u if u < chunk_steps else u - chunk_steps) * BL
                # chain: psum <- A^T @ alpha ; alpha' <- psum * em[:, step]
                mm = nc.tensor.matmul(ps[:], a_sb[:], cur[:])
                if u % renorm_every != 1:
                    # A is already in the PE array (loaded by the preamble
                    # matmul / the self-loading matmul after each renorm);
                    # skip the per-step LDWEIGHTS to keep it off the chain.
                    mm.ins.ldweights = False
                nc.vector.tensor_mul(nxt[:], ps[:], emt[:, c0:c0 + BL])

                if u % renorm_every == 0:
                    # Z = colsum of the alpha just produced (off-chain-ish)
                    m = u // renorm_every
                    nc.tensor.matmul(zps[:], ones_sb[:], nxt[:])
                    nc.vector.reciprocal(rz[:], zps[:])
                    nc.scalar.copy(
                        zring[0:1, bass.ds((it * n_renorm_per_body + m) * BL,
                                           BL)],
                        zps[0:1, :])
                if u % renorm_every == renorm_lag:
                    # lagged rescale: alpha *= 1/Z (broadcast over partitions)
                    nc.vector.tensor_mul(nxt[:], nxt[:], rz[:])

                if u == chunk_steps - 1:
                    # em0's readers are done; prefetch em0 <- chunk 2i+2
                    for s in range(n_dma_slices):
                        off = s * slice_cols
                        nc.sync.dma_start(
                            em0[:, off:off + slice_cols],
                            em_d[:, bass.ds(base + 2 * chunk_cols + off,
                                            slice_cols)])

        # steps_per_body is even, so the final alpha is in al_a
        nc.sync.dma_start(afin_d[:], al_a[:])
        nc.sync.dma_start(zbuf_d[:], zring[:])

    nc.compile()
    return nc


def host_prepare(obs, I, A, Bm, n_iters=None):
    """Shard + precompute per-core device inputs and host bookkeeping."""
    if n_iters is None:
        n_iters = T_FULL // STEPS_PER_BODY
    T = n_iters * STEPS_PER_BODY
    chunk_cols = CHUNK_STEPS * BL
    em_cols = (2 * n_iters + 1) * chunk_cols

    obs = np.asarray(obs)
    I64 = np.asarray(I, np.float64)
    A64 = np.asarray(A, np.float64)
    Bm64 = np.asarray(Bm, np.float64)

    # stationary distribution of A -> per-symbol magnitude predictor
    pi = np.full(P, 1.0 / P)
    for _ in range(300):
        pi = pi @ A64
    f_sym = pi @ Bm64                                   # [E]
    Bmh32 = (Bm64 / f_sym[None, :]).astype(np.float32)  # folded emissions

    if MM_DTYPE == "bfloat16":
        import ml_dtypes
        mm_np = ml_dtypes.bfloat16
    else:
        mm_np = np.float32
    ones = np.ones((P, P), mm_np)
    A32 = np.asarray(A, np.float32).astype(mm_np)

    in_maps = []
    book = []
    for c in range(N_CORES):
        ob = obs[c * BL:(c + 1) * BL]                   # [16, T_total]
        # step 0 on host (float64): alpha0 = I * Bm[:, obs0], normalized
        a0 = I64[:, None] * Bm64[:, ob[:, 0]]           # [S, 16]
        Z0 = a0.sum(0)
        alpha0 = (a0 / Z0).astype(np.float32).astype(mm_np)
        # emission stream: col j <-> chain step j+1 for j < T-1;
        # col T-1 is all-ones (sum-preserving extra step); pad chunk zeros.
        em = np.empty((P, em_cols), np.float32)
        obsT = ob[:, 1:T].T                             # [T-1, 16]
        em[:, :(T - 1) * BL] = Bmh32[:, obsT.reshape(-1)]
        em[:, (T - 1) * BL:T * BL] = 1.0
        em[:, T * BL:] = 0.0
        in_maps.append({"em": em, "amat": A32, "ones": ones,
                        "alpha0": alpha0})
        # host-side fp64 pieces of the final loglik
        cnt = np.stack([(ob[:, 1:T] == e).sum(1) for e in range(E_SYM)], 1)
        ll_base = np.log(Z0) + (cnt * np.log(f_sym)[None, :]).sum(1)  # [16]
        book.append(ll_base)
    return in_maps, book


def assemble_output(results, book):
    """Combine device outputs + host bookkeeping into loglik [128] f32."""
    out = np.empty(B_FULL, np.float64)
    for c in range(N_CORES):
        afin = results[c]["alpha_fin"].astype(np.float64)   # [S, 16]
        zb = results[c]["zbuf"].astype(np.float64).reshape(-1, BL)  # [R, 16]
        ll = book[c] + np.log(zb).sum(0) + np.log(afin.sum(0))
        out[c * BL:(c + 1) * BL] = ll
    return out.astype(np.float32)


_NC_CACHE = {}


def _get_nc(n_iters):
    key = n_iters
    if key not in _NC_CACHE:
        _NC_CACHE[key] = build_nc(n_iters, STEPS_PER_BODY, CHUNK_STEPS,
                                  RENORM_EVERY, RENORM_LAG)
    return _NC_CACHE[key]


def kernel(obs, I, A, Bm):
    from concourse.bass_utils import run_bass_kernel_spmd

    n_iters = T_FULL // STEPS_PER_BODY
    nc = _get_nc(n_iters)
    in_maps, book = host_prepare(obs, I, A, Bm, n_iters)
    res = run_bass_kernel_spmd(nc, in_maps, core_ids=list(range(N_CORES)))
    return assemble_output(res.results, book)


# revision 11
# speedup vs baseline: 2.0080x; 1.0101x over previous
"""Trainium2 Bass kernel for the CgpHmm scaled-forward layer.

Computes loglik[b] = scaled HMM forward log-likelihood over B=128 sequences
of length T=8192 with S=128 hidden states and an alphabet of E=6 symbols.

Strategy (data-parallel over batch, 16 sequences per core on 8 cores):
  - Keep alphaT [S=128 partitions, 16 seqs] resident in SBUF.
  - Per time step: PE matmul (stationary A, 16 moving cols) computes
    A^T @ alphaT into PSUM; one DVE tensor_mul multiplies by the emission
    column (streamed from HBM) and writes the next alphaT back to SBUF.
    This 2-instruction serial chain (8192 links) is latency-bound and is
    the entire kernel critical path.
  - Emission probabilities are pre-divided on the host by a per-symbol
    predictor f_sym[e] = (stationary dist of A) @ Bm[:, e] so that alpha's
    magnitude drifts slowly; exact per-sequence renormalization happens
    every 128 steps via a ones-vector matmul (Z), reciprocal, and a lagged
    rescale 16 steps later.  All log bookkeeping (sum of log Z, log f
    counts, final log-sum) happens on the host in float64.
"""

import os
import sys

import numpy as np

sys.path.insert(0, "/opt/trn_rl_repo")

P = 128          # states / partitions
BL = 16          # sequences per core
N_CORES = 8
B_FULL = 128
T_FULL = 8192
E_SYM = 6

# device schedule constants (full size)
STEPS_PER_BODY = 1024     # chain steps per For_i iteration
CHUNK_STEPS = 512         # steps per emission SBUF tile (2 tiles per body)
RENORM_EVERY = 128
RENORM_LAG = 16
MM_DTYPE = "bfloat16"     # matmul operand dtype ("float32" | "bfloat16")


def build_nc(n_iters, steps_per_body, chunk_steps, renorm_every, renorm_lag,
             debug=False):
    """Build the per-core Bass program (identical for all cores)."""
    import concourse.bacc as bacc
    import concourse.bass as bass
    import concourse.mybir as mybir
    import concourse.tile as tile

    assert steps_per_body == 2 * chunk_steps
    assert steps_per_body % renorm_every == 0
    n_renorm_per_body = steps_per_body // renorm_every
    n_renorms = n_iters * n_renorm_per_body
    chunk_cols = chunk_steps * BL
    # chunks used: 2*n_iters, plus one pad chunk for the final em0 prefetch
    em_cols = (2 * n_iters + 1) * chunk_cols

    nc = bacc.Bacc(None, target_bir_lowering=False, debug=debug)

    f32 = mybir.dt.float32
    mdt = getattr(mybir.dt, MM_DTYPE)
    em_d = nc.dram_tensor("em", [P, em_cols], f32, kind="ExternalInput")
    a_d = nc.dram_tensor("amat", [P, P], mdt, kind="ExternalInput")
    ones_d = nc.dram_tensor("ones", [P, P], mdt, kind="ExternalInput")
    alpha0_d = nc.dram_tensor("alpha0", [P, BL], mdt, kind="ExternalInput")
    afin_d = nc.dram_tensor("alpha_fin", [P, BL], mdt, kind="ExternalOutput")
    zbuf_d = nc.dram_tensor("zbuf", [1, max(n_renorms, 1) * BL], f32,
                            kind="ExternalOutput")

    with tile.TileContext(nc) as tc, \
            tc.tile_pool(name="sb", bufs=1) as sbp, \
            tc.tile_pool(name="ps", bufs=1, space="PSUM") as psp:
        a_sb = sbp.tile([P, P], mdt, name="a_sb")
        ones_sb = sbp.tile([P, P], mdt, name="ones_sb")
        al_a = sbp.tile([P, BL], mdt, name="al_a")
        al_b = sbp.tile([P, BL], mdt, name="al_b")
        em0 = sbp.tile([P, chunk_cols], f32, name="em0")
        em1 = sbp.tile([P, chunk_cols], f32, name="em1")
        rz = sbp.tile([P, BL], f32, name="rz")
        zring = sbp.tile([1, max(n_renorms, 1) * BL], f32, name="zring")
        pp0 = psp.tile([P, BL], f32, name="pp0")
        pp1 = psp.tile([P, BL], f32, name="pp1")
        zps = psp.tile([P, BL], f32, name="zps")

        # preamble loads
        nc.sync.dma_start(a_sb[:], a_d[:])
        nc.sync.dma_start(ones_sb[:], ones_d[:])
        nc.sync.dma_start(al_a[:], alpha0_d[:])
        # chunk 0 preload (body 0 reads it as em0)
        n_dma_slices = 8
        slice_cols = chunk_cols // n_dma_slices
        for s in range(n_dma_slices):
            nc.sync.dma_start(em0[:, s * slice_cols:(s + 1) * slice_cols],
                              em_d[:, s * slice_cols:(s + 1) * slice_cols])

        alphas = (al_a, al_b)
        psums = (pp0, pp1)

        # establish A as the loaded stationary operand (result discarded;
        # zps is overwritten by the first renorm matmul before any read)
        nc.tensor.matmul(zps[:], a_sb[:], al_a[:])

        hint = (mybir.EngineType.PE, mybir.EngineType.DVE,
                mybir.EngineType.SP, mybir.EngineType.Activation)
        with tc.For_i(0, n_iters, 1, hint_engines=hint) as it:
            # prefetch em1 <- chunk 2i+1 (read in this body's second half)
            base = it * (2 * chunk_cols)
            for s in range(n_dma_slices):
                off = s * slice_cols
                nc.sync.dma_start(
                    em1[:, off:off + slice_cols],
                    em_d[:, bass.ds(base + chunk_cols + off, slice_cols)])

            for u in range(steps_per_body):
                cur = alphas[u % 2]
                nxt = alphas[(u + 1) % 2]
                ps = psums[u % 2]
                emt = em0 if u < chunk_steps else em1
                c0 = (u if u < chunk_steps else u - chunk_steps) * BL
                # chain: psum <- A^T @ alpha ; alpha' <- psum * em[:, step]
                mm = nc.tensor.matmul(ps[:], a_sb[:], cur[:])
                if u % renorm_every != 1:
                    # A is already in the PE array (loaded by the preamble
                    # matmul / the self-loading matmul after each renorm);
                    # skip the per-step LDWEIGHTS to keep it off the chain.
                    mm.ins.ldweights = False
                nc.vector.tensor_mul(nxt[:], ps[:], emt[:, c0:c0 + BL])

                if u % renorm_every == 0:
                    # Z = colsum of the alpha just produced (off-chain-ish)
                    m = u // renorm_every
                    nc.tensor.matmul(zps[:], ones_sb[:], nxt[:])
                    nc.vector.reciprocal(rz[:], zps[:])
                    nc.scalar.copy(
                        zring[0:1, bass.ds((it * n_renorm_per_body + m) * BL,
                                           BL)],
                        zps[0:1, :])
                if u % renorm_every == renorm_lag:
                    # lagged rescale: alpha *= 1/Z (broadcast over partitions)
                    nc.vector.tensor_mul(nxt[:], nxt[:], rz[:])

                if u == chunk_steps - 1:
                    # em0's readers are done; prefetch em0 <- chunk 2i+2
                    for s in range(n_dma_slices):
                        off = s * slice_cols
                        nc.sync.dma_start(
                            em0[:, off:off + slice_cols],
                            em_d[:, bass.ds(base + 2 * chunk_cols + off,
                                            slice_cols)])

        # steps_per_body is even, so the final alpha is in al_a
        nc.sync.dma_start(afin_d[:], al_a[:])
        nc.sync.dma_start(zbuf_d[:], zring[:])

    nc.compile()
    return nc


def host_prepare(obs, I, A, Bm, n_iters=None):
    """Shard + precompute per-core device inputs and host bookkeeping."""
    if n_iters is None:
        n_iters = T_FULL // STEPS_PER_BODY
    T = n_iters * STEPS_PER_BODY
    chunk_cols = CHUNK_STEPS * BL
    em_cols = (2 * n_iters + 1) * chunk_cols

    obs = np.asarray(obs)
    I64 = np.asarray(I, np.float64)
    A64 = np.asarray(A, np.float64)
    Bm64 = np.asarray(Bm, np.float64)

    # stationary distribution of A -> per-symbol magnitude predictor
    pi = np.full(P, 1.0 / P)
    for _ in range(300):
        pi = pi @ A64
    f_sym = pi @ Bm64                                   # [E]
    Bmh32 = (Bm64 / f_sym[None, :]).astype(np.float32)  # folded emissions

    if MM_DTYPE == "bfloat16":
        import ml_dtypes
        mm_np = ml_dtypes.bfloat16
    else:
        mm_np = np.float32
    ones = np.ones((P, P), mm_np)
    A32 = np.asarray(A, np.float32).astype(mm_np)

    in_maps = []
    book = []
    for c in range(N_CORES):
        ob = obs[c * BL:(c + 1) * BL]                   # [16, T_total]
        # step 0 on host (float64): alpha0 = I * Bm[:, obs0], normalized
        a0 = I64[:, None] * Bm64[:, ob[:, 0]]           # [S, 16]
        Z0 = a0.sum(0)
        alpha0 = (a0 / Z0).astype(np.float32).astype(mm_np)
        # emission stream: col j <-> chain step j+1 for j < T-1;
        # col T-1 is all-ones (sum-preserving extra step); pad chunk zeros.
        em = np.empty((P, em_cols), np.float32)
        obsT = ob[:, 1:T].T                             # [T-1, 16]
        em[:, :(T - 1) * BL] = Bmh32[:, obsT.reshape(-1)]
        em[:, (T - 1) * BL:T * BL] = 1.0
        em[:, T * BL:] = 0.0
        in_maps.append({"em": em, "amat": A32, "ones": ones,
                        "alpha0": alpha0})
        # host-side fp64 pieces of the final loglik
        cnt = np.stack([(ob[:, 1:T] == e).sum(1) for e in range(E_SYM)], 1)
        ll_base = np.log(Z0) + (cnt * np.log(f_sym)[None, :]).sum(1)  # [16]
        book.append(ll_base)
    return in_maps, book


def assemble_output(results, book):
    """Combine device outputs + host bookkeeping into loglik [128] f32."""
    out = np.empty(B_FULL, np.float64)
    for c in range(N_CORES):
        afin = results[c]["alpha_fin"].astype(np.float64)   # [S, 16]
        zb = results[c]["zbuf"].astype(np.float64).reshape(-1, BL)  # [R, 16]
        ll = book[c] + np.log(zb).sum(0) + np.log(afin.sum(0))
        out[c * BL:(c + 1) * BL] = ll
    return out.astype(np.float32)


_NC_CACHE = {}


def _get_nc(n_iters):
    key = n_iters
    if key not in _NC_CACHE:
        _NC_CACHE[key] = build_nc(n_iters, STEPS_PER_BODY, CHUNK_STEPS,
                                  RENORM_EVERY, RENORM_LAG)
    return _NC_CACHE[key]


def kernel(obs, I, A, Bm):
    from concourse.bass_utils import run_bass_kernel_spmd

    n_iters = T_FULL // STEPS_PER_BODY
    nc = _get_nc(n_iters)
    in_maps, book = host_prepare(obs, I, A, Bm, n_iters)
    res = run_bass_kernel_spmd(nc, in_maps, core_ids=list(range(N_CORES)))
    return assemble_output(res.results, book)
